# revision 30
# baseline (speedup 1.0000x reference)
"""Trainium2 Bass kernel for nn_DotAttention (B=8 data-parallel over 8 cores).

Per core (one batch element):
  xp = relu(x @ Wi + bi)            [2048, 96]
  mp = relu(m @ Wm + bm)            [2048, 96]
  S.T[jm, jx] = mp[jm,:] . xp[jx,:]             (PE, fp16 operands)
  E = exp(S.T / sqrt(96) + maskbias[jm])        (ACT, mask folded into bias)
  U.T[d, jx] = sum_jm mtilde[jm, d] * E[jm, jx] (PE; mtilde = [m | 1] so row 150
                                                 of U.T is the softmax denom)
  out.T = sigmoid(Wg.T @ res.T + bg) * res.T,  res.T = [x.T ; U.T / denom]
  out = PE-transpose(out.T)  -> DMA

All matmul operands are fp16 (PSUM accumulation is fp32); data paths that
reach the output directly (x, U, gates multiplied by res) stay fp32.

Everything is transposed on-chip ("T layout": feature dim on partitions)
because the PE contracts over the partition dim; x/m are transposed on entry
via PE-transpose, the output is transposed back at the end.
"""

import math

import numpy as np

import concourse.bass as bass
import concourse.mybir as mybir
import concourse.tile as tile
from concourse import bacc
from concourse.bass_utils import run_bass_kernel_spmd
from concourse.masks import make_identity

F32 = mybir.dt.float32
F16 = mybir.dt.float16
I32 = mybir.dt.int32

B = 8
JX = 2048
JM = 2048
D = 150
H = 96
G = 300
NCH = JX // 128  # 16 column chunks of the transposed layout
NJT = JM // 128  # 16 key tiles
HALF = 1024  # jx processed in two halves (PSUM budget)
NSUB = HALF // 512  # matmul free-dim is limited to 512 (one PSUM bank)
SCALE = 1.0 / math.sqrt(float(H))
NEG_BIG = 1.0e30


def _body(tc, x_d, m_d, mask_d, wi_d, bi_d, wm_d, bm_d, wg_d, bg_d, o_d):
    nc = tc.nc
    Relu = mybir.ActivationFunctionType.Relu
    Exp = mybir.ActivationFunctionType.Exp
    Sigmoid = mybir.ActivationFunctionType.Sigmoid

    import contextlib

    with contextlib.ExitStack() as ctx:
        const = ctx.enter_context(tc.tile_pool(name="const", bufs=1))
        work = ctx.enter_context(tc.tile_pool(name="work", bufs=2))
        epool = ctx.enter_context(tc.tile_pool(name="epool", bufs=6))
        psb = ctx.enter_context(tc.tile_pool(name="psb", bufs=2, space="PSUM"))
        psu = ctx.enter_context(tc.tile_pool(name="psu", bufs=1, space="PSUM"))

        ident = const.tile([128, 128], F32)
        make_identity(nc, ident)
        ident16 = const.tile([128, 128], F16)
        make_identity(nc, ident16)

        # HAM warm-up: ~3.4us of sustained matmul activity lifts the PE
        # clock gate from 1.2 to 2.4 GHz. The burst runs while the input
        # DMAs land (PE would be idle anyway); keep-warm singles are
        # sprinkled through the transpose phase (transposes don't count as
        # PE activity for the HAM window).
        def warm_mm(n):
            # one tile for the whole burst: per-MM tile allocation would
            # serialize on slot releases and spread the burst out
            jp = psb.tile([128, 128], F32, tag="big", name="junk")
            for _ in range(n):
                nc.tensor.matmul(
                    jp, ident16, ident16, start=True, stop=True,
                    skip_group_check=True,
                )
        warm_mm(32)

        # ---- input loads -------------------------------------------------
        # mtilde in fp16, natural layout (jm on partitions); columns are
        # [m | 0-pad | 1]. Engine APs must start at a 32-aligned partition,
        # so the all-ones column (softmax denominator) is padded out to
        # column 160 -> U2 partition 32. Loaded first (gates m transposes
        # and the attention stationaries); split into chunks so transposes
        # can start before the whole tensor lands.
        mt16 = const.tile([128, NJT, 162], F16)
        m_nat = const.tile([128, NJT, D], F32)
        m_re = m_d.rearrange("(n p) d -> p n d", p=128)
        for q in range(8):
            qs = slice(q * NJT // 8, (q + 1) * NJT // 8)
            nc.scalar.dma_start(out=m_nat[:, qs, :], in_=m_re[:, qs, :])
            nc.vector.tensor_copy(out=mt16[:, qs, 0:D], in_=m_nat[:, qs, :])
        nc.vector.memset(mt16[:, :, D:160], 0.0)
        nc.vector.memset(mt16[:, :, 160:161], 1.0)
        x_nat = const.tile([128, NCH, D], F32)
        x_re = x_d.rearrange("(n p) d -> p n d", p=128)
        for q in range(8):
            qs = slice(q * NCH // 8, (q + 1) * NCH // 8)
            nc.sync.dma_start(out=x_nat[:, qs, :], in_=x_re[:, qs, :])

        # mask -> per-partition additive bias for exp: (mask-1)*1e30
        mask_sb = const.tile([NJT, 128], I32)
        nc.sync.dma_start(out=mask_sb, in_=mask_d.rearrange("(n p) -> n p", p=128))
        maskf = const.tile([NJT, 128], F32)
        nc.vector.tensor_copy(out=maskf, in_=mask_sb)
        nc.vector.tensor_scalar(
            out=maskf,
            in0=maskf,
            scalar1=1.0,
            scalar2=NEG_BIG,
            op0=mybir.AluOpType.subtract,
            op1=mybir.AluOpType.mult,
        )
        mb_ps = psb.tile([128, NJT], F32, tag="big")
        nc.tensor.transpose(mb_ps, maskf, ident[:NJT, :NJT])
        maskbias = const.tile([128, NJT], F32)
        nc.vector.tensor_copy(out=maskbias, in_=mb_ps)

        # ---- weights -----------------------------------------------------
        wstage = const.tile([128, 2 * H], F32)
        nc.sync.dma_start(out=wstage[:, 0:H], in_=wi_d[0:128, :])
        nc.sync.dma_start(out=wstage[:, H : 2 * H], in_=wm_d[0:128, :])
        wstage2 = const.tile([D - 128, 2 * H], F32)
        nc.sync.dma_start(out=wstage2[:, 0:H], in_=wi_d[128:D, :])
        nc.sync.dma_start(out=wstage2[:, H : 2 * H], in_=wm_d[128:D, :])
        wi16a = const.tile([128, H], F16)
        nc.vector.tensor_copy(out=wi16a, in_=wstage[:, 0:H])
        wi16b = const.tile([D - 128, H], F16)
        nc.vector.tensor_copy(out=wi16b, in_=wstage2[:, 0:H])
        wm16a = const.tile([128, H], F16)
        nc.vector.tensor_copy(out=wm16a, in_=wstage[:, H : 2 * H])
        wm16b = const.tile([D - 128, H], F16)
        nc.vector.tensor_copy(out=wm16b, in_=wstage2[:, H : 2 * H])
        bi_sb = const.tile([H, 1], F32)
        nc.sync.dma_start(out=bi_sb, in_=bi_d.rearrange("(n one) -> n one", one=1))
        bm_sb = const.tile([H, 1], F32)
        nc.sync.dma_start(out=bm_sb, in_=bm_d.rearrange("(n one) -> n one", one=1))
        wg16 = []
        for gi, (g0, g1) in enumerate([(0, 128), (128, D), (D, D + 128), (D + 128, G)]):
            wst = const.tile(
                [g1 - g0, G], F32, tag=f"wgst_{gi}", name=f"wgst_{gi}"
            )
            nc.sync.dma_start(out=wst, in_=wg_d[g0:g1, :])
            w = const.tile([g1 - g0, G], F16, tag=f"wg16_{gi}", name=f"wg16_{gi}")
            nc.vector.tensor_copy(out=w, in_=wst)
            wg16.append(w)
        bg_sb = []
        for gi, (g0, g1) in enumerate([(0, 128), (128, D), (D, D + 128), (D + 128, G)]):
            t = const.tile([g1 - g0, 1], F32, tag=f"bg_{gi}", name=f"bg_{gi}")
            nc.sync.dma_start(
                out=t, in_=bg_d[g0:g1].rearrange("(n one) -> n one", one=1)
            )
            bg_sb.append(t)
        ones16 = const.tile([1, 128], F16)
        nc.vector.memset(ones16, 1.0)

        # ---- transpose x and m into T layout -----------------------------
        xT1 = const.tile([128, JX], F32)
        xT2 = const.tile([D - 128, JX], F32)
        xT116 = const.tile([128, JX], F16)
        xT216 = const.tile([D - 128, JX], F16)
        mT116 = const.tile([128, JM], F16)
        mT216 = const.tile([D - 128, JM], F16)
        def tp_tile(i, shape, dt, name):
            kind = ["big", "big", "u1", "u2"][i % 4]
            pool = psb if kind == "big" else psu
            return pool.tile(shape, dt, tag=kind, name=name)

        ti = 0
        for c in range(NCH):
            sl = slice(c * 128, (c + 1) * 128)
            eng = nc.vector if c % 2 == 0 else nc.scalar
            t1 = tp_tile(ti, [128, 128], F32, "xtp1"); ti += 1
            nc.tensor.transpose(t1, x_nat[:, c, 0:128], ident)
            (eng.tensor_copy if c % 2 == 0 else eng.copy)(out=xT1[:, sl], in_=t1)
            t2 = tp_tile(ti, [D - 128, 128], F32, "xtp2"); ti += 1
            nc.tensor.transpose(t2, x_nat[:, c, 128:D], ident)
            (eng.tensor_copy if c % 2 == 0 else eng.copy)(out=xT2[:, sl], in_=t2)
        # bulk fp16 casts instead of 64 small per-chunk copies
        nc.vector.tensor_copy(out=xT116, in_=xT1)
        nc.vector.tensor_copy(out=xT216, in_=xT2)
        for c in range(NJT):
            sl = slice(c * 128, (c + 1) * 128)
            eng = nc.vector if c % 2 == 0 else nc.scalar
            t1 = tp_tile(ti, [128, 128], F16, "mtp1"); ti += 1
            nc.tensor.transpose(t1, mt16[:, c, 0:128], ident16)
            (eng.tensor_copy if c % 2 == 0 else eng.copy)(out=mT116[:, sl], in_=t1)
            t2 = tp_tile(ti, [D - 128, 128], F16, "mtp2"); ti += 1
            nc.tensor.transpose(t2, mt16[:, c, 128:D], ident16)
            (eng.tensor_copy if c % 2 == 0 else eng.copy)(out=mT216[:, sl], in_=t2)

        warm_mm(24)

        # ---- projections: xpT = relu(Wi.T @ x.T + bi), same for m --------
        xpT16 = const.tile([H, JX], F16)
        mpT16 = const.tile([H, JM], F16)
        for wa, wb, bsb, srcA, srcB, dst in [
            (wi16a, wi16b, bi_sb, xT116, xT216, xpT16),
            (wm16a, wm16b, bm_sb, mT116, mT216, mpT16),
        ]:
            for h in range(2):
                hs = slice(h * HALF, (h + 1) * HALF)
                pp = psb.tile([H, HALF], F32, tag="big")
                for s in range(NSUB):
                    ss = slice(h * HALF + s * 512, h * HALF + (s + 1) * 512)
                    ps = slice(s * 512, (s + 1) * 512)
                    nc.tensor.matmul(
                        pp[:, ps], wa, srcA[:, ss],
                        start=True, stop=False, skip_group_check=True,
                    )
                    nc.tensor.matmul(
                        pp[:, ps], wb, srcB[:, ss],
                        start=False, stop=True, skip_group_check=True,
                    )
                nc.scalar.activation(
                    out=dst[:, hs], in_=pp, func=Relu, bias=bsb, scale=1.0
                )

        # ---- attention: scores -> exp -> weighted sum, per jx half -------
        # then normalize, gate, transpose back and store, still per half so
        # half-0 tail work overlaps half-1 attention on other engines.
        o_re = o_d.rearrange("(n p) k -> n p k", p=128)
        kranges = [(0, 128), (128, D), (D, D + 128), (D + 128, G)]
        U1n, U2n, rr16n = [], [], []
        for h in range(2):
            hs = slice(h * HALF, (h + 1) * HALF)
            U1 = psu.tile([128, HALF], F32, tag="u1")
            U2 = psu.tile([33, HALF], F32, tag="u2")
            for j in range(NJT):
                sp = psb.tile([128, HALF], F32, tag="big")
                for s in range(NSUB):
                    ss = slice(h * HALF + s * 512, h * HALF + (s + 1) * 512)
                    nc.tensor.matmul(
                        sp[:, s * 512 : (s + 1) * 512],
                        mpT16[:, j * 128 : (j + 1) * 128],
                        xpT16[:, ss],
                        start=True, stop=True, skip_group_check=True,
                    )
                e16 = epool.tile([128, HALF], F16, tag="e16")
                nc.scalar.activation(
                    out=e16, in_=sp, func=Exp,
                    bias=maskbias[:, j : j + 1], scale=SCALE,
                )
                for s in range(NSUB):
                    ps = slice(s * 512, (s + 1) * 512)
                    nc.tensor.matmul(
                        U1[:, ps], mt16[:, j, 0:128], e16[:, ps],
                        start=(j == 0), stop=(j == NJT - 1), skip_group_check=True,
                    )
                    nc.tensor.matmul(
                        U2[:, ps], mt16[:, j, 128:161], e16[:, ps],
                        start=(j == 0), stop=(j == NJT - 1), skip_group_check=True,
                    )
            # norm head: stage U in SBUF + reciprocal of the denominator.
            # No PE instructions here — the PE queue is FIFO and must flow
            # straight into the next half's attention matmuls.
            U1c = work.tile([128, HALF], F32, tag="U1c")
            nc.vector.tensor_copy(out=U1c, in_=U1)
            U2c = work.tile([33, HALF], F32, tag="U2c")
            nc.vector.tensor_copy(out=U2c, in_=U2)
            U1n.append(U1c)
            U2n.append(U2c)
            if h == 0:
                # h0's reciprocal runs on DVE while h1's attention occupies
                # the PE; h1's is emitted at the end of h0's tail so it does
                # not block h0's normalization muls in the DVE FIFO
                rr = work.tile([1, HALF], F32, tag="rr")
                nc.vector.reciprocal(out=rr, in_=U2c[32:33, :])
                rr16 = work.tile([1, HALF], F16, tag="rr16")
                nc.vector.tensor_copy(out=rr16, in_=rr)
                rr16n.append(rr16)

        # ---- gating + store, after both attention halves ------------------
        # (the PE queue is FIFO: tails must come after all attention matmuls
        # so the normalization chains overlap attention instead of stalling).
        # Both the contraction (g) and output (k) dims use the four
        # partition-aligned chunks [0:128],[128:150],[150:278],[278:300] so
        # x.T and U feed the matmul and the gate multiply with no
        # partition-shifting DMAs at all.
        warm_mm(28)
        for h in range(2):
            hs = slice(h * HALF, (h + 1) * HALF)
            # norm tail: PE broadcast of 1/denom, then normalize U
            bc = psb.tile([128, HALF], F32, tag="big")
            for sx in range(NSUB):
                ps = slice(sx * 512, (sx + 1) * 512)
                nc.tensor.matmul(
                    bc[:, ps], ones16, rr16n[h][:, ps],
                    start=True, stop=True, skip_group_check=True,
                )
            RCraw = work.tile([128, HALF], F32, tag="RCraw")
            nc.vector.tensor_mul(out=RCraw, in0=U1n[h], in1=bc)
            RDraw = work.tile([D - 128, HALF], F32, tag="RDraw")
            nc.vector.tensor_mul(
                out=RDraw, in0=U2n[h][0 : D - 128, :], in1=bc[0 : D - 128, :]
            )
            RC16 = work.tile([128, HALF], F16, tag="RC16")
            nc.vector.tensor_copy(out=RC16, in_=RCraw)
            RD16 = work.tile([D - 128, HALF], F16, tag="RD16")
            nc.vector.tensor_copy(out=RD16, in_=RDraw)
            if h == 0:
                rr = work.tile([1, HALF], F32, tag="rr")
                nc.vector.reciprocal(out=rr, in_=U2n[1][32:33, :])
                rr16 = work.tile([1, HALF], F16, tag="rr16")
                nc.vector.tensor_copy(out=rr16, in_=rr)
                rr16n.append(rr16)
            res16 = [xT116[:, hs], xT216[:, hs], RC16, RD16]
            resf = [xT1[:, hs], xT2[:, hs], RCraw, RDraw]
            oT = [
                work.tile([128, HALF], F32, tag="oT0", name="oT0"),
                work.tile([D - 128, HALF], F32, tag="oT1", name="oT1"),
                work.tile([128, HALF], F32, tag="oT2", name="oT2"),
                work.tile([D - 128, HALF], F32, tag="oT3", name="oT3"),
            ]
            for kc, (k0, k1) in enumerate(kranges):
                kw = k1 - k0
                gp_kind = ["u1", "u2"][kc % 2]
                gp = psu.tile([kw, HALF], F32, tag=gp_kind, name="gp")
                for sx in range(NSUB):
                    ps = slice(sx * 512, (sx + 1) * 512)
                    for gc in range(4):
                        nc.tensor.matmul(
                            gp[:, ps], wg16[gc][:, k0:k1], res16[gc][:, ps],
                            start=(gc == 0), stop=(gc == 3),
                            skip_group_check=True,
                        )
                gs = work.tile([kw, HALF], F32, tag="gs", bufs=3)
                nc.scalar.activation(
                    out=gs, in_=gp, func=Sigmoid, bias=bg_sb[kc], scale=1.0
                )
                nc.vector.tensor_mul(out=oT[kc], in0=gs, in1=resf[kc])

            for c in range(NCH // 2):
                sl = slice(c * 128, (c + 1) * 128)
                op = psb.tile([128, 320], F32, tag="big")
                nc.tensor.transpose(op[:, 0:128], oT[0][:, sl], ident)
                nc.tensor.transpose(
                    op[:, 128:D], oT[1][:, sl], ident[: D - 128, : D - 128]
                )
                nc.tensor.transpose(op[:, D : D + 128], oT[2][:, sl], ident)
                nc.tensor.transpose(
                    op[:, D + 128 : G], oT[3][:, sl], ident[: D - 128, : D - 128]
                )
                onat = work.tile([128, G], F32, tag="onat", bufs=3)
                nc.vector.tensor_copy(out=onat, in_=op[:, 0:G])
                nc.sync.dma_start(out=o_re[h * (NCH // 2) + c], in_=onat)


_NC_CACHE = None


def _build_nc():
    global _NC_CACHE
    if _NC_CACHE is not None:
        return _NC_CACHE
    nc = bacc.Bacc(None, target_bir_lowering=False, debug=False)
    x_d = nc.dram_tensor("x", [JX, D], F32, kind="ExternalInput")
    m_d = nc.dram_tensor("m", [JM, D], F32, kind="ExternalInput")
    mask_d = nc.dram_tensor("mask", [JM], I32, kind="ExternalInput")
    wi_d = nc.dram_tensor("Wi", [D, H], F32, kind="ExternalInput")
    bi_d = nc.dram_tensor("bi", [H], F32, kind="ExternalInput")
    wm_d = nc.dram_tensor("Wm", [D, H], F32, kind="ExternalInput")
    bm_d = nc.dram_tensor("bm", [H], F32, kind="ExternalInput")
    wg_d = nc.dram_tensor("Wg", [G, G], F32, kind="ExternalInput")
    bg_d = nc.dram_tensor("bg", [G], F32, kind="ExternalInput")
    o_d = nc.dram_tensor("out", [JX, G], F32, kind="ExternalOutput")
    with tile.TileContext(nc) as tc:
        _body(tc, x_d, m_d, mask_d, wi_d, bi_d, wm_d, bm_d, wg_d, bg_d, o_d)
    nc.finalize()
    _NC_CACHE = nc
    return nc


def _in_maps(inputs, memory, mask, Wi, bi, Wm, bm, Wg, bg):
    maps = []
    for b in range(B):
        maps.append(
            {
                "x": np.ascontiguousarray(inputs[b], dtype=np.float32),
                "m": np.ascontiguousarray(memory[b], dtype=np.float32),
                "mask": np.ascontiguousarray(mask[b], dtype=np.int32),
                "Wi": np.ascontiguousarray(Wi, dtype=np.float32),
                "bi": np.ascontiguousarray(bi, dtype=np.float32),
                "Wm": np.ascontiguousarray(Wm, dtype=np.float32),
                "bm": np.ascontiguousarray(bm, dtype=np.float32),
                "Wg": np.ascontiguousarray(Wg, dtype=np.float32),
                "bg": np.ascontiguousarray(bg, dtype=np.float32),
            }
        )
    return maps


def run_spmd(inputs, memory, mask, Wi, bi, Wm, bm, Wg, bg, **spmd_kwargs):
    """Run the kernel across 8 cores; returns the BassKernelResults."""
    nc = _build_nc()
    maps = _in_maps(
        np.asarray(inputs), np.asarray(memory), np.asarray(mask),
        np.asarray(Wi), np.asarray(bi), np.asarray(Wm), np.asarray(bm),
        np.asarray(Wg), np.asarray(bg),
    )
    return run_bass_kernel_spmd(nc, maps, list(range(B)), **spmd_kwargs)


def kernel(inputs, memory, mask, Wi, bi, Wm, bm, Wg, bg):
    res = run_spmd(inputs, memory, mask, Wi, bi, Wm, bm, Wg, bg)
    out = np.stack([res.results[b]["out"] for b in range(B)], axis=0)
    return out.astype(np.float32)
